# revision 21
# baseline (speedup 1.0000x reference)
"""Grouped MoE (top-2 of 8 experts, SwiGLU) on 8 Trainium2 NeuronCores.

Expert-parallel with host routing (gate on host, exact). Core c owns
expert c; tokens are gathered per expert into a fixed-capacity [D, cap]
buffer. On device each core runs the three SwiGLU GEMMs in bf16 over its
~T*K/E tokens and writes an UNSCALED output in transposed [D, cap]
layout; the host applies the per-token gate weight and scatter-adds the
two expert contributions. No collectives.

v2 layout/schedule changes vs the first working kernel:
 - All DRAM inputs are packed partition-major so every DMA moves 2-16 KB
   contiguous rows (128 descriptors/transfer, full HBM rate). w1/w3 are
   packed in per-f-tile blocks so the first A-stage matmul only needs
   x-chunk0 + one 256 KB block instead of ~2 MB.
 - Y-stage is w2-stationary (output [D, tokens]): no partial m-tiles,
   arbitrary chunk sizes, and the gate scale moves to the host combine.
 - First/last chunks are small to shrink the DMA lead-in and the
   copy+store tail after the final matmul.
"""

import sys
import numpy as np

for _p in ("/opt/trn_rl_repo",):
    if _p not in sys.path:
        sys.path.insert(0, _p)

B, S, D, F, E, K = 2, 2048, 1024, 1024, 8, 2
T = B * S            # 4096 tokens
NCORES = 8
P = 128
DK = D // P          # 8 contraction chunks over D
FK = F // P          # 8 F tiles
BLK = DK * P         # w1/w3 f-block stride (k-major within a block)
NWARM = 12           # PE warm-up matmuls while the first DMAs land

_cache = {}


def _chunks(cap):
    """Token chunks <= 512 (PSUM bank limit), first/last kept small-ish.

    A/B and Y matmul cost is proportional to total tokens for any chunk
    >= ~192 (LDWEIGHTS hides under the column stream), so only the first
    chunk (gates the DMA lead-in) and last chunk (gates the tail) matter.
    """
    if cap <= 512:
        sizes = [cap]
    elif cap <= 832:
        sizes = [(cap + 1) // 2, cap // 2]
    else:
        sizes = [320]
        rem = cap - 320
        while rem > 704:
            sizes.append(512)
            rem -= 512
        if rem > 512:
            sizes += [(rem + 1) // 2, rem // 2]
        else:
            sizes.append(rem)
    out = []
    o = 0
    for s in sizes:
        out.append((o, s))
        o += s
    assert o == cap and all(0 < s <= 512 for _, s in out)
    return out


def _build_nc(cap):
    from contextlib import ExitStack

    import concourse.mybir as mybir
    import concourse.tile as tile
    from concourse import bacc

    dt = mybir.dt
    AF = mybir.ActivationFunctionType
    ALU = mybir.AluOpType

    chunks = _chunks(cap)

    nc = bacc.Bacc("TRN2", target_bir_lowering=False, debug=False,
                   num_devices=NCORES)

    # all partition-major: row p holds that partition's full data span
    xh = nc.dram_tensor("xh", [P, DK * cap], dt.bfloat16,
                        kind="ExternalInput").ap()
    w1h = nc.dram_tensor("w1h", [P, FK * BLK], dt.bfloat16,
                         kind="ExternalInput").ap()
    w3h = nc.dram_tensor("w3h", [P, FK * BLK], dt.bfloat16,
                         kind="ExternalInput").ap()
    w2h = nc.dram_tensor("w2h", [P, FK * D], dt.bfloat16,
                         kind="ExternalInput").ap()
    # out is packed [(chunk)(dhalf)(dtile)(tok)] so every store DMA moves
    # one [128, 4*tcz] tile with 1.8-4KB contiguous rows (the per-dtile
    # layout would fragment rows to 456B and go descriptor-rate-bound);
    # the host de-interleaves during combine.
    out = nc.dram_tensor("out", [P, DK * cap], dt.bfloat16,
                         kind="ExternalOutput").ap()

    with tile.TileContext(nc) as tc, ExitStack() as ctx:
        # two pools total (per-tag bufs overrides) — each pool costs
        # alloc/release sync chains across every engine at kernel start/end
        sb = ctx.enter_context(tc.tile_pool(name="sb", bufs=1))
        ps = ctx.enter_context(tc.tile_pool(name="ps", bufs=2, space="PSUM"))

        x_sb = sb.tile([P, DK * cap], dt.bfloat16, tag="xall")
        w1_sb = sb.tile([P, FK * BLK], dt.bfloat16, tag="w1")
        w3_sb = sb.tile([P, FK * BLK], dt.bfloat16, tag="w3")
        w2_sb = sb.tile([P, FK * D], dt.bfloat16, tag="w2")

        # ---- DMA issue order = criticality order. Every transfer below is
        # 128 descriptors of >= 2KB contiguous rows (full HBM rate).
        # First A-group (f=0) is gated only on x-chunk0 + w1/w3 f0 blocks
        # (~1 MB); remaining f-blocks stream in ahead of the compute. ----
        # All DMA on the sync queue: its sequencer starts issuing at ~0.1us
        # while the other engines only come online at 5-8us, so anything
        # issued elsewhere would land BEHIND these transfers in the rings.
        o0, tc0 = chunks[0]
        nc.sync.dma_start(x_sb[:, 0:DK * tc0], xh[:, 0:DK * tc0])
        nc.sync.dma_start(w1_sb[:, 0:BLK], w1h[:, 0:BLK])
        nc.sync.dma_start(w3_sb[:, 0:BLK], w3h[:, 0:BLK])
        for f in range(1, FK):
            nc.sync.dma_start(w1_sb[:, f * BLK:(f + 1) * BLK],
                              w1h[:, f * BLK:(f + 1) * BLK])
            nc.sync.dma_start(w3_sb[:, f * BLK:(f + 1) * BLK],
                              w3h[:, f * BLK:(f + 1) * BLK])
        # w2 in fk-halves; the Y loop consumes fk 0..3 before 4..7
        nc.sync.dma_start(w2_sb[:, 0:4 * D], w2h[:, 0:4 * D])
        nc.sync.dma_start(w2_sb[:, 4 * D:8 * D], w2h[:, 4 * D:8 * D])
        for (o, tcz) in chunks[1:]:
            nc.sync.dma_start(x_sb[:, DK * o:DK * (o + tcz)],
                              xh[:, DK * o:DK * (o + tcz)])

        # ---- PE warm-up: dummy matmuls while the first DMAs land keep the
        # HAM activity window full so the PE reaches max p-state ----
        wrm = sb.tile([P, 512], dt.bfloat16, tag="wrm")
        nc.vector.memset(wrm[:], 0.5)
        for _ in range(NWARM):
            psW = ps.tile([P, 512], dt.float32, tag="psA", name="psW")
            nc.tensor.matmul(psW[:], lhsT=wrm[:, 0:P], rhs=wrm[:],
                             start=True, stop=True)

        # ---- per-chunk SwiGLU FFN ----
        for (o, tcz) in chunks:
            xo = DK * o
            h_sb = []
            for f in range(FK):
                psA = ps.tile([P, tcz], dt.float32, tag="psA")
                for k in range(DK):
                    nc.tensor.matmul(
                        psA[:], lhsT=w1_sb[:, f * BLK + k * P:f * BLK + (k + 1) * P],
                        rhs=x_sb[:, xo + k * tcz:xo + (k + 1) * tcz],
                        start=(k == 0), stop=(k == DK - 1))
                psB = ps.tile([P, tcz], dt.float32, tag="psB")
                for k in range(DK):
                    nc.tensor.matmul(
                        psB[:], lhsT=w3_sb[:, f * BLK + k * P:f * BLK + (k + 1) * P],
                        rhs=x_sb[:, xo + k * tcz:xo + (k + 1) * tcz],
                        start=(k == 0), stop=(k == DK - 1))
                ssb = sb.tile([P, tcz], dt.bfloat16, tag="ssb", bufs=2)
                nc.scalar.activation(ssb[:], psA[:], AF.Silu)
                hsb = sb.tile([P, tcz], dt.bfloat16, tag=f"h{f}", bufs=2)
                nc.vector.tensor_tensor(hsb[:], ssb[:], psB[:], op=ALU.mult)
                h_sb.append(hsb)
            # Y-stage, w2-stationary: psY[dt] = sum_fk w2T[fk, dtile] @ h[fk]
            # fkh-outer so the first half only needs w2 cols 0..4D
            for dhalf in range(2):
                psY = [ps.tile([P, tcz], dt.float32, tag="psY", bufs=4,
                                  name=f"psY{dhalf}_{i}") for i in range(4)]
                for fkh in range(2):
                    for dt_ in range(4):
                        dglob = dhalf * 4 + dt_
                        for fk in range(fkh * 4, fkh * 4 + 4):
                            nc.tensor.matmul(
                                psY[dt_][:],
                                lhsT=w2_sb[:, fk * D + dglob * P:fk * D + dglob * P + P],
                                rhs=h_sb[fk][:],
                                start=(fk == 0), stop=(fk == FK - 1))
                ysb = sb.tile([P, 4 * tcz], dt.bfloat16, tag="ysb", bufs=3)
                for dt_ in range(4):
                    # alternate copy engines: two parallel copy streams
                    dst = ysb[:, dt_ * tcz:(dt_ + 1) * tcz]
                    if dt_ % 2 == 0:
                        nc.scalar.activation(dst, psY[dt_][:], AF.Copy)
                    else:
                        nc.vector.tensor_scalar_mul(dst, psY[dt_][:], 1.0)
                base = DK * o + dhalf * 4 * tcz
                if o + tcz == cap and dhalf == 1:
                    # final store in halves: dt0/dt1 finish ~1us before the
                    # last matmul, so their half overlaps the stream end and
                    # only a 0.12MB transfer rides the critical tail
                    nc.sync.dma_start(out[:, base:base + 2 * tcz],
                                      ysb[:, 0:2 * tcz])
                    nc.sync.dma_start(out[:, base + 2 * tcz:base + 4 * tcz],
                                      ysb[:, 2 * tcz:4 * tcz])
                else:
                    nc.sync.dma_start(out[:, base:base + 4 * tcz], ysb[:])

    nc.compile()
    return nc


def _route(xf, gate_w):
    """Host gate: returns per-expert (token indices, renormalized weights)."""
    logits = xf.astype(np.float64) @ gate_w.astype(np.float64).T   # [T, E]
    order = np.argsort(-logits, axis=1, kind="stable")
    i1 = order[:, 0]
    i2 = order[:, 1]
    ar = np.arange(T)
    l1 = logits[ar, i1]
    l2 = logits[ar, i2]
    g1 = 1.0 / (1.0 + np.exp(l2 - l1))
    g2 = 1.0 - g1
    idx_e, scl_e = [], []
    for e in range(E):
        m1 = i1 == e
        m2 = i2 == e
        ids = np.concatenate([np.nonzero(m1)[0], np.nonzero(m2)[0]])
        sc = np.concatenate([g1[m1], g2[m2]])
        idx_e.append(ids)
        scl_e.append(sc.astype(np.float32))
    return idx_e, scl_e


def prepare(x, gate_w, w1, w3, w2):
    """Host routing + sharding: returns (nc, in_maps, (idx_e, scl_e))."""
    import ml_dtypes

    xf = np.ascontiguousarray(x.reshape(T, D).astype(np.float32))
    xTb = np.ascontiguousarray(xf.T).astype(ml_dtypes.bfloat16)   # [D, T]

    idx_e, scl_e = _route(xf, gate_w)
    maxcnt = max(len(i) for i in idx_e)
    cap = ((maxcnt + 3) // 4) * 4     # 4-token grain keeps DMA rows 8B-aligned
    chunks = _chunks(cap)

    if cap not in _cache:
        _cache[cap] = _build_nc(cap)
    nc = _cache[cap]

    in_maps = []
    for c in range(NCORES):
        ids = idx_e[c]
        cnt = len(ids)
        xg = np.zeros((D, cap), dtype=ml_dtypes.bfloat16)
        xg[:, :cnt] = xTb[:, ids]
        # chunk-major, then k-major partition blocks: chunk rows contiguous
        xh = np.concatenate([
            xg[:, o:o + tcz].reshape(DK, P, tcz).transpose(1, 0, 2)
            .reshape(P, DK * tcz) for (o, tcz) in chunks], axis=1)

        w1T = np.ascontiguousarray(w1[c].T).astype(ml_dtypes.bfloat16)  # [D,F]
        w3T = np.ascontiguousarray(w3[c].T).astype(ml_dtypes.bfloat16)
        w2T = np.ascontiguousarray(w2[c].T).astype(ml_dtypes.bfloat16)  # [F,D]

        def fmaj(wT):
            # [D, F] -> [128, f-major [f][k][128]] per-f-tile blocks
            return np.concatenate([
                wT[:, f * P:(f + 1) * P].reshape(DK, P, P).transpose(1, 0, 2)
                .reshape(P, BLK) for f in range(FK)], axis=1)

        in_maps.append({
            "xh": np.ascontiguousarray(xh),
            "w1h": fmaj(w1T),
            "w3h": fmaj(w3T),
            "w2h": np.ascontiguousarray(
                w2T.reshape(FK, P, D).transpose(1, 0, 2).reshape(P, FK * D)),
        })
    return nc, in_maps, (idx_e, scl_e, chunks)


def _combine(res, meta):
    idx_e, scl_e, chunks = meta
    outf = np.zeros((T, D), dtype=np.float32)
    for c in range(NCORES):
        cnt = len(idx_e[c])
        raw = res.results[c]["out"].astype(np.float32)   # [128, 8*cap]
        cap = raw.shape[1] // DK
        y = np.empty((D, cap), dtype=np.float32)
        for (o, tcz) in chunks:
            blk = raw[:, DK * o:DK * (o + tcz)].reshape(P, 8, tcz)
            for dglob in range(8):
                y[dglob * P:(dglob + 1) * P, o:o + tcz] = blk[:, dglob, :]
        outf[idx_e[c]] += scl_e[c][:, None] * y[:, :cnt].T
    return outf.reshape(B, S, D)


def kernel(x, gate_w, w1, w3, w2):
    from concourse.bass_utils import run_bass_kernel_spmd

    nc, in_maps, meta = prepare(x, gate_w, w1, w3, w2)
    res = run_bass_kernel_spmd(nc, in_maps, list(range(NCORES)))
    return _combine(res, meta)


# revision 22
# speedup vs baseline: 1.0024x; 1.0024x over previous
"""Grouped MoE (top-2 of 8 experts, SwiGLU) on 8 Trainium2 NeuronCores.

Expert-parallel with host routing (gate on host, exact). Core c owns
expert c; tokens are gathered per expert into a fixed-capacity [D, cap]
buffer. On device each core runs the three SwiGLU GEMMs in bf16 over its
~T*K/E tokens and writes an UNSCALED output in a packed D-transposed
layout; the host de-interleaves, applies the per-token gate weight and
scatter-adds the two expert contributions. No collectives.

Layout/schedule design (all trace-driven; see the HAM warning below):
 - All DRAM tensors are packed partition-major so every DMA moves
   0.9-16 KB contiguous rows (~128 descriptors/transfer, full HBM rate;
   the per-queue descriptor rate makes <512B rows the bottleneck
   otherwise). w1/w3 are packed in per-f-tile blocks so the first
   A-stage matmul is gated on ~0.9 MB instead of ~2 MB.
 - Y-stage is w2-stationary (output [D-tile, tokens]): no partial
   m-tiles, arbitrary chunk sizes, gate scale moves to the host.
 - Stores batch 4 D-tiles into one [128, 4*tcz] tile per (chunk, half);
   the very last store goes in two halves so only ~0.12 MB rides the
   critical tail after the final matmul.
 - The 12 x 512-col PE warm-up is LOAD-BEARING: the HAM latches the max
   p-state only after ~4-5us of continuous PE activity ending with ZERO
   idle gap into the real stream. Shorter warm-ups or any gap latch
   ~2.0 GHz instead of ~2.35 GHz for the WHOLE run (+17us). Warm-up
   overshooting the DMA-ready moment is protective, not waste.
 - All dma_starts stay on nc.sync: its sequencer issues at ~0.1us while
   other engines only come online at 5-8us.
"""

import sys
import numpy as np

for _p in ("/opt/trn_rl_repo",):
    if _p not in sys.path:
        sys.path.insert(0, _p)

B, S, D, F, E, K = 2, 2048, 1024, 1024, 8, 2
T = B * S            # 4096 tokens
NCORES = 8
P = 128
DK = D // P          # 8 contraction chunks over D
FK = F // P          # 8 F tiles
BLK = DK * P         # w1/w3 f-block stride (k-major within a block)
NWARM = 12           # PE warm-up matmuls while the first DMAs land

_cache = {}


def _chunks(cap):
    """Token chunks <= 512 (PSUM bank limit), first/last kept small-ish.

    A/B and Y matmul cost is proportional to total tokens for any chunk
    >= ~192 (LDWEIGHTS hides under the column stream), so only the first
    chunk (gates the DMA lead-in) and last chunk (gates the tail) matter.
    """
    if cap <= 512:
        sizes = [cap]
    elif cap <= 832:
        sizes = [(cap + 1) // 2, cap // 2]
    else:
        sizes = [320]
        rem = cap - 320
        while rem > 704:
            sizes.append(512)
            rem -= 512
        if rem > 512:
            sizes += [(rem + 1) // 2, rem // 2]
        else:
            sizes.append(rem)
    out = []
    o = 0
    for s in sizes:
        out.append((o, s))
        o += s
    assert o == cap and all(0 < s <= 512 for _, s in out)
    return out


def _build_nc(cap):
    from contextlib import ExitStack

    import concourse.mybir as mybir
    import concourse.tile as tile
    from concourse import bacc

    dt = mybir.dt
    AF = mybir.ActivationFunctionType
    ALU = mybir.AluOpType

    chunks = _chunks(cap)

    nc = bacc.Bacc("TRN2", target_bir_lowering=False, debug=False,
                   num_devices=NCORES)

    # all partition-major: row p holds that partition's full data span
    xh = nc.dram_tensor("xh", [P, DK * cap], dt.bfloat16,
                        kind="ExternalInput").ap()
    w1h = nc.dram_tensor("w1h", [P, FK * BLK], dt.bfloat16,
                         kind="ExternalInput").ap()
    w3h = nc.dram_tensor("w3h", [P, FK * BLK], dt.bfloat16,
                         kind="ExternalInput").ap()
    w2h = nc.dram_tensor("w2h", [P, FK * D], dt.bfloat16,
                         kind="ExternalInput").ap()
    # out is packed [(chunk)(dhalf)(dtile)(tok)] so every store DMA moves
    # one [128, 4*tcz] tile with 1.8-4KB contiguous rows (the per-dtile
    # layout would fragment rows to 456B and go descriptor-rate-bound);
    # the host de-interleaves during combine.
    out = nc.dram_tensor("out", [P, DK * cap], dt.bfloat16,
                         kind="ExternalOutput").ap()

    with tile.TileContext(nc) as tc, ExitStack() as ctx:
        # two pools total (per-tag bufs overrides) — each pool costs
        # alloc/release sync chains across every engine at kernel start/end
        sb = ctx.enter_context(tc.tile_pool(name="sb", bufs=1))
        ps = ctx.enter_context(tc.tile_pool(name="ps", bufs=2, space="PSUM"))

        x_sb = sb.tile([P, DK * cap], dt.bfloat16, tag="xall")
        w1_sb = sb.tile([P, FK * BLK], dt.bfloat16, tag="w1")
        w3_sb = sb.tile([P, FK * BLK], dt.bfloat16, tag="w3")
        w2_sb = sb.tile([P, FK * D], dt.bfloat16, tag="w2")

        # ---- DMA issue order = criticality order. Every transfer below is
        # 128 descriptors of >= 2KB contiguous rows (full HBM rate).
        # First A-group (f=0) is gated only on x-chunk0 + w1/w3 f0 blocks
        # (~1 MB); remaining f-blocks stream in ahead of the compute. ----
        # All DMA on the sync queue: its sequencer starts issuing at ~0.1us
        # while the other engines only come online at 5-8us, so anything
        # issued elsewhere would land BEHIND these transfers in the rings.
        o0, tc0 = chunks[0]
        nc.sync.dma_start(x_sb[:, 0:DK * tc0], xh[:, 0:DK * tc0])
        nc.sync.dma_start(w1_sb[:, 0:BLK], w1h[:, 0:BLK])
        nc.sync.dma_start(w3_sb[:, 0:BLK], w3h[:, 0:BLK])
        for f in range(1, FK):
            nc.sync.dma_start(w1_sb[:, f * BLK:(f + 1) * BLK],
                              w1h[:, f * BLK:(f + 1) * BLK])
            nc.sync.dma_start(w3_sb[:, f * BLK:(f + 1) * BLK],
                              w3h[:, f * BLK:(f + 1) * BLK])
        # w2 in fk-halves; the Y loop consumes fk 0..3 before 4..7
        nc.sync.dma_start(w2_sb[:, 0:4 * D], w2h[:, 0:4 * D])
        nc.sync.dma_start(w2_sb[:, 4 * D:8 * D], w2h[:, 4 * D:8 * D])
        for (o, tcz) in chunks[1:]:
            nc.sync.dma_start(x_sb[:, DK * o:DK * (o + tcz)],
                              xh[:, DK * o:DK * (o + tcz)])

        # ---- PE warm-up: dummy matmuls while the first DMAs land keep the
        # HAM activity window full so the PE reaches max p-state ----
        wrm = sb.tile([P, 512], dt.bfloat16, tag="wrm")
        nc.vector.memset(wrm[:], 0.5)
        for _ in range(NWARM):
            psW = ps.tile([P, 512], dt.float32, tag="psA", name="psW")
            nc.tensor.matmul(psW[:], lhsT=wrm[:, 0:P], rhs=wrm[:],
                             start=True, stop=True)

        # ---- per-chunk SwiGLU FFN ----
        for (o, tcz) in chunks:
            xo = DK * o
            h_sb = []
            for f in range(FK):
                psA = ps.tile([P, tcz], dt.float32, tag="psA")
                for k in range(DK):
                    nc.tensor.matmul(
                        psA[:], lhsT=w1_sb[:, f * BLK + k * P:f * BLK + (k + 1) * P],
                        rhs=x_sb[:, xo + k * tcz:xo + (k + 1) * tcz],
                        start=(k == 0), stop=(k == DK - 1))
                psB = ps.tile([P, tcz], dt.float32, tag="psB")
                for k in range(DK):
                    nc.tensor.matmul(
                        psB[:], lhsT=w3_sb[:, f * BLK + k * P:f * BLK + (k + 1) * P],
                        rhs=x_sb[:, xo + k * tcz:xo + (k + 1) * tcz],
                        start=(k == 0), stop=(k == DK - 1))
                ssb = sb.tile([P, tcz], dt.bfloat16, tag="ssb", bufs=2)
                nc.scalar.activation(ssb[:], psA[:], AF.Silu)
                hsb = sb.tile([P, tcz], dt.bfloat16, tag=f"h{f}", bufs=2)
                nc.vector.tensor_tensor(hsb[:], ssb[:], psB[:], op=ALU.mult)
                h_sb.append(hsb)
            # Y-stage, w2-stationary: psY[dt] = sum_fk w2T[fk, dtile] @ h[fk]
            # fkh-outer so the first half only needs w2 cols 0..4D
            for dhalf in range(2):
                psY = [ps.tile([P, tcz], dt.float32, tag="psY", bufs=4,
                                  name=f"psY{dhalf}_{i}") for i in range(4)]
                for fkh in range(2):
                    for dt_ in range(4):
                        dglob = dhalf * 4 + dt_
                        for fk in range(fkh * 4, fkh * 4 + 4):
                            nc.tensor.matmul(
                                psY[dt_][:],
                                lhsT=w2_sb[:, fk * D + dglob * P:fk * D + dglob * P + P],
                                rhs=h_sb[fk][:],
                                start=(fk == 0), stop=(fk == FK - 1))
                ysb = sb.tile([P, 4 * tcz], dt.bfloat16, tag="ysb", bufs=3)
                for dt_ in range(4):
                    # alternate copy engines: two parallel copy streams
                    dst = ysb[:, dt_ * tcz:(dt_ + 1) * tcz]
                    if dt_ % 2 == 0:
                        nc.scalar.activation(dst, psY[dt_][:], AF.Copy)
                    else:
                        nc.vector.tensor_scalar_mul(dst, psY[dt_][:], 1.0)
                base = DK * o + dhalf * 4 * tcz
                if o + tcz == cap and dhalf == 1:
                    # final store in halves: dt0/dt1 finish ~1us before the
                    # last matmul, so their half overlaps the stream end and
                    # only a 0.12MB transfer rides the critical tail
                    nc.sync.dma_start(out[:, base:base + 2 * tcz],
                                      ysb[:, 0:2 * tcz])
                    nc.sync.dma_start(out[:, base + 2 * tcz:base + 4 * tcz],
                                      ysb[:, 2 * tcz:4 * tcz])
                else:
                    nc.sync.dma_start(out[:, base:base + 4 * tcz], ysb[:])

    nc.compile()
    return nc


def _route(xf, gate_w):
    """Host gate: returns per-expert (token indices, renormalized weights)."""
    logits = xf.astype(np.float64) @ gate_w.astype(np.float64).T   # [T, E]
    order = np.argsort(-logits, axis=1, kind="stable")
    i1 = order[:, 0]
    i2 = order[:, 1]
    ar = np.arange(T)
    l1 = logits[ar, i1]
    l2 = logits[ar, i2]
    g1 = 1.0 / (1.0 + np.exp(l2 - l1))
    g2 = 1.0 - g1
    idx_e, scl_e = [], []
    for e in range(E):
        m1 = i1 == e
        m2 = i2 == e
        ids = np.concatenate([np.nonzero(m1)[0], np.nonzero(m2)[0]])
        sc = np.concatenate([g1[m1], g2[m2]])
        idx_e.append(ids)
        scl_e.append(sc.astype(np.float32))
    return idx_e, scl_e


def prepare(x, gate_w, w1, w3, w2):
    """Host routing + sharding: returns (nc, in_maps, (idx_e, scl_e))."""
    import ml_dtypes

    xf = np.ascontiguousarray(x.reshape(T, D).astype(np.float32))
    xTb = np.ascontiguousarray(xf.T).astype(ml_dtypes.bfloat16)   # [D, T]

    idx_e, scl_e = _route(xf, gate_w)
    maxcnt = max(len(i) for i in idx_e)
    cap = ((maxcnt + 3) // 4) * 4     # 4-token grain keeps DMA rows 8B-aligned
    chunks = _chunks(cap)

    if cap not in _cache:
        _cache[cap] = _build_nc(cap)
    nc = _cache[cap]

    in_maps = []
    for c in range(NCORES):
        ids = idx_e[c]
        cnt = len(ids)
        xg = np.zeros((D, cap), dtype=ml_dtypes.bfloat16)
        xg[:, :cnt] = xTb[:, ids]
        # chunk-major, then k-major partition blocks: chunk rows contiguous
        xh = np.concatenate([
            xg[:, o:o + tcz].reshape(DK, P, tcz).transpose(1, 0, 2)
            .reshape(P, DK * tcz) for (o, tcz) in chunks], axis=1)

        w1T = np.ascontiguousarray(w1[c].T).astype(ml_dtypes.bfloat16)  # [D,F]
        w3T = np.ascontiguousarray(w3[c].T).astype(ml_dtypes.bfloat16)
        w2T = np.ascontiguousarray(w2[c].T).astype(ml_dtypes.bfloat16)  # [F,D]

        def fmaj(wT):
            # [D, F] -> [128, f-major [f][k][128]] per-f-tile blocks
            return np.concatenate([
                wT[:, f * P:(f + 1) * P].reshape(DK, P, P).transpose(1, 0, 2)
                .reshape(P, BLK) for f in range(FK)], axis=1)

        in_maps.append({
            "xh": np.ascontiguousarray(xh),
            "w1h": fmaj(w1T),
            "w3h": fmaj(w3T),
            "w2h": np.ascontiguousarray(
                w2T.reshape(FK, P, D).transpose(1, 0, 2).reshape(P, FK * D)),
        })
    return nc, in_maps, (idx_e, scl_e, chunks)


def _combine(res, meta):
    idx_e, scl_e, chunks = meta
    outf = np.zeros((T, D), dtype=np.float32)
    for c in range(NCORES):
        cnt = len(idx_e[c])
        raw = res.results[c]["out"].astype(np.float32)   # [128, 8*cap]
        cap = raw.shape[1] // DK
        y = np.empty((D, cap), dtype=np.float32)
        for (o, tcz) in chunks:
            blk = raw[:, DK * o:DK * (o + tcz)].reshape(P, 8, tcz)
            for dglob in range(8):
                y[dglob * P:(dglob + 1) * P, o:o + tcz] = blk[:, dglob, :]
        outf[idx_e[c]] += scl_e[c][:, None] * y[:, :cnt].T
    return outf.reshape(B, S, D)


def kernel(x, gate_w, w1, w3, w2):
    from concourse.bass_utils import run_bass_kernel_spmd

    nc, in_maps, meta = prepare(x, gate_w, w1, w3, w2)
    res = run_bass_kernel_spmd(nc, in_maps, list(range(NCORES)))
    return _combine(res, meta)


# revision 23
# speedup vs baseline: 1.0039x; 1.0015x over previous
"""Grouped MoE (top-2 of 8 experts, SwiGLU) on 8 Trainium2 NeuronCores.

Expert-parallel with host routing (gate on host, exact). Core c owns
expert c; tokens are gathered per expert into a fixed-capacity [D, cap]
buffer. On device each core runs the three SwiGLU GEMMs in bf16 over its
~T*K/E tokens and writes an UNSCALED output in a packed D-transposed
layout; the host de-interleaves, applies the per-token gate weight and
scatter-adds the two expert contributions. No collectives.

Layout/schedule design (all trace-driven; see the HAM warning below):
 - All DRAM tensors are packed partition-major so every DMA moves
   0.9-16 KB contiguous rows (~128 descriptors/transfer, full HBM rate;
   the per-queue descriptor rate makes <512B rows the bottleneck
   otherwise). w1/w3 are packed in per-f-tile blocks so the first
   A-stage matmul is gated on ~0.9 MB instead of ~2 MB.
 - Y-stage is w2-stationary (output [D-tile, tokens]): no partial
   m-tiles, arbitrary chunk sizes, gate scale moves to the host.
 - Stores batch 4 D-tiles into one [128, 4*tcz] tile per (chunk, half);
   the very last store goes in two halves so only ~0.12 MB rides the
   critical tail after the final matmul.
 - The 12 x 512-col PE warm-up is LOAD-BEARING: the HAM latches the max
   p-state only after ~4-5us of continuous PE activity ending with ZERO
   idle gap into the real stream. Shorter warm-ups or any gap latch
   ~2.0 GHz instead of ~2.35 GHz for the WHOLE run (+17us). Warm-up
   overshooting the DMA-ready moment is protective, not waste.
 - All dma_starts stay on nc.sync: its sequencer issues at ~0.1us while
   other engines only come online at 5-8us.
"""

import sys
import numpy as np

for _p in ("/opt/trn_rl_repo",):
    if _p not in sys.path:
        sys.path.insert(0, _p)

B, S, D, F, E, K = 2, 2048, 1024, 1024, 8, 2
T = B * S            # 4096 tokens
NCORES = 8
P = 128
DK = D // P          # 8 contraction chunks over D
FK = F // P          # 8 F tiles
BLK = DK * P         # w1/w3 f-block stride (k-major within a block)
NWARM = 12           # PE warm-up matmuls while the first DMAs land

_cache = {}


def _chunks(cap):
    """Token chunks <= 512 (PSUM bank limit), first/last kept small-ish.

    A/B and Y matmul cost is proportional to total tokens for any chunk
    >= ~192 (LDWEIGHTS hides under the column stream), so only the first
    chunk (gates the DMA lead-in) and last chunk (gates the tail) matter.
    """
    if cap <= 512:
        sizes = [cap]
    elif cap <= 832:
        sizes = [(cap + 1) // 2, cap // 2]
    else:
        sizes = [320]
        rem = cap - 320
        while rem > 704:
            sizes.append(512)
            rem -= 512
        if rem > 512:
            sizes += [(rem + 1) // 2, rem // 2]
        else:
            sizes.append(rem)
    out = []
    o = 0
    for s in sizes:
        out.append((o, s))
        o += s
    assert o == cap and all(0 < s <= 512 for _, s in out)
    return out


def _build_nc(cap):
    from contextlib import ExitStack

    import concourse.mybir as mybir
    import concourse.tile as tile
    from concourse import bacc

    dt = mybir.dt
    AF = mybir.ActivationFunctionType
    ALU = mybir.AluOpType

    chunks = _chunks(cap)

    nc = bacc.Bacc("TRN2", target_bir_lowering=False, debug=False,
                   num_devices=NCORES)

    # all partition-major: row p holds that partition's full data span
    xh = nc.dram_tensor("xh", [P, DK * cap], dt.bfloat16,
                        kind="ExternalInput").ap()
    w1h = nc.dram_tensor("w1h", [P, FK * BLK], dt.bfloat16,
                         kind="ExternalInput").ap()
    w3h = nc.dram_tensor("w3h", [P, FK * BLK], dt.bfloat16,
                         kind="ExternalInput").ap()
    w2h = nc.dram_tensor("w2h", [P, FK * D], dt.bfloat16,
                         kind="ExternalInput").ap()
    # out is packed [(chunk)(dhalf)(dtile)(tok)] so every store DMA moves
    # one [128, 4*tcz] tile with 1.8-4KB contiguous rows (the per-dtile
    # layout would fragment rows to 456B and go descriptor-rate-bound);
    # the host de-interleaves during combine.
    out = nc.dram_tensor("out", [P, DK * cap], dt.bfloat16,
                         kind="ExternalOutput").ap()

    with tile.TileContext(nc) as tc, ExitStack() as ctx:
        # two pools total (per-tag bufs overrides) — each pool costs
        # alloc/release sync chains across every engine at kernel start/end
        sb = ctx.enter_context(tc.tile_pool(name="sb", bufs=1))
        ps = ctx.enter_context(tc.tile_pool(name="ps", bufs=2, space="PSUM"))

        x_sb = sb.tile([P, DK * cap], dt.bfloat16, tag="xall")
        w1_sb = sb.tile([P, FK * BLK], dt.bfloat16, tag="w1")
        w3_sb = sb.tile([P, FK * BLK], dt.bfloat16, tag="w3")
        w2_sb = sb.tile([P, FK * D], dt.bfloat16, tag="w2")

        # ---- DMA issue order = criticality order. Every transfer below is
        # 128 descriptors of >= 2KB contiguous rows (full HBM rate).
        # First A-group (f=0) is gated only on x-chunk0 + w1/w3 f0 blocks
        # (~1 MB); remaining f-blocks stream in ahead of the compute. ----
        # All DMA on the sync queue: its sequencer starts issuing at ~0.1us
        # while the other engines only come online at 5-8us, so anything
        # issued elsewhere would land BEHIND these transfers in the rings.
        o0, tc0 = chunks[0]
        nc.sync.dma_start(x_sb[:, 0:DK * tc0], xh[:, 0:DK * tc0])
        nc.sync.dma_start(w1_sb[:, 0:BLK], w1h[:, 0:BLK])
        nc.sync.dma_start(w3_sb[:, 0:BLK], w3h[:, 0:BLK])
        for f in range(1, FK):
            nc.sync.dma_start(w1_sb[:, f * BLK:(f + 1) * BLK],
                              w1h[:, f * BLK:(f + 1) * BLK])
            nc.sync.dma_start(w3_sb[:, f * BLK:(f + 1) * BLK],
                              w3h[:, f * BLK:(f + 1) * BLK])
        # w2 in fk-halves; the Y loop consumes fk 0..3 before 4..7
        nc.sync.dma_start(w2_sb[:, 0:4 * D], w2h[:, 0:4 * D])
        nc.sync.dma_start(w2_sb[:, 4 * D:8 * D], w2h[:, 4 * D:8 * D])
        for (o, tcz) in chunks[1:]:
            nc.sync.dma_start(x_sb[:, DK * o:DK * (o + tcz)],
                              xh[:, DK * o:DK * (o + tcz)])

        # ---- PE warm-up: dummy matmuls while the first DMAs land keep the
        # HAM activity window full so the PE reaches max p-state ----
        wrm = sb.tile([P, 512], dt.bfloat16, tag="wrm")
        nc.vector.memset(wrm[:], 0.5)
        for _ in range(NWARM):
            psW = ps.tile([P, 512], dt.float32, tag="psA", name="psW")
            nc.tensor.matmul(psW[:], lhsT=wrm[:, 0:P], rhs=wrm[:],
                             start=True, stop=True)

        # ---- per-chunk SwiGLU FFN ----
        for (o, tcz) in chunks:
            xo = DK * o
            h_sb = []
            for f in range(FK):
                psA = ps.tile([P, tcz], dt.float32, tag="psA")
                for k in range(DK):
                    nc.tensor.matmul(
                        psA[:], lhsT=w1_sb[:, f * BLK + k * P:f * BLK + (k + 1) * P],
                        rhs=x_sb[:, xo + k * tcz:xo + (k + 1) * tcz],
                        start=(k == 0), stop=(k == DK - 1))
                psB = ps.tile([P, tcz], dt.float32, tag="psB")
                for k in range(DK):
                    nc.tensor.matmul(
                        psB[:], lhsT=w3_sb[:, f * BLK + k * P:f * BLK + (k + 1) * P],
                        rhs=x_sb[:, xo + k * tcz:xo + (k + 1) * tcz],
                        start=(k == 0), stop=(k == DK - 1))
                ssb = sb.tile([P, tcz], dt.bfloat16, tag="ssb", bufs=2)
                nc.scalar.activation(ssb[:], psA[:], AF.Silu)
                hsb = sb.tile([P, tcz], dt.bfloat16, tag=f"h{f}", bufs=2)
                nc.vector.tensor_tensor(hsb[:], ssb[:], psB[:], op=ALU.mult)
                h_sb.append(hsb)
            # Y-stage, w2-stationary: psY[dt] = sum_fk w2T[fk, dtile] @ h[fk]
            # fkh-outer so the first half only needs w2 cols 0..4D
            for dhalf in range(2):
                psY = [ps.tile([P, tcz], dt.float32, tag="psY", bufs=4,
                                  name=f"psY{dhalf}_{i}") for i in range(4)]
                for fkh in range(2):
                    for dt_ in range(4):
                        dglob = dhalf * 4 + dt_
                        for fk in range(fkh * 4, fkh * 4 + 4):
                            nc.tensor.matmul(
                                psY[dt_][:],
                                lhsT=w2_sb[:, fk * D + dglob * P:fk * D + dglob * P + P],
                                rhs=h_sb[fk][:],
                                start=(fk == 0), stop=(fk == FK - 1))
                ysb = sb.tile([P, 4 * tcz], dt.bfloat16, tag="ysb", bufs=3)
                fin = o + tcz == cap and dhalf == 1
                for dt_ in range(4):
                    # alternate copy engines: two parallel copy streams
                    dst = ysb[:, dt_ * tcz:(dt_ + 1) * tcz]
                    if fin and dt_ == 3:
                        # the very last copy is the post-stream critical
                        # chain: halve it across both engines
                        hw_ = tcz // 2
                        nc.scalar.activation(dst[:, 0:hw_],
                                             psY[dt_][:, 0:hw_], AF.Copy)
                        nc.vector.tensor_scalar_mul(dst[:, hw_:tcz],
                                                    psY[dt_][:, hw_:tcz], 1.0)
                    elif dt_ % 2 == 0:
                        nc.scalar.activation(dst, psY[dt_][:], AF.Copy)
                    else:
                        nc.vector.tensor_scalar_mul(dst, psY[dt_][:], 1.0)
                base = DK * o + dhalf * 4 * tcz
                if o + tcz == cap and dhalf == 1:
                    # final store in halves: dt0/dt1 finish ~1us before the
                    # last matmul, so their half overlaps the stream end and
                    # only a 0.12MB transfer rides the critical tail
                    nc.sync.dma_start(out[:, base:base + 2 * tcz],
                                      ysb[:, 0:2 * tcz])
                    nc.sync.dma_start(out[:, base + 2 * tcz:base + 4 * tcz],
                                      ysb[:, 2 * tcz:4 * tcz])
                else:
                    nc.sync.dma_start(out[:, base:base + 4 * tcz], ysb[:])

    nc.compile()
    return nc


def _route(xf, gate_w):
    """Host gate: returns per-expert (token indices, renormalized weights)."""
    logits = xf.astype(np.float64) @ gate_w.astype(np.float64).T   # [T, E]
    order = np.argsort(-logits, axis=1, kind="stable")
    i1 = order[:, 0]
    i2 = order[:, 1]
    ar = np.arange(T)
    l1 = logits[ar, i1]
    l2 = logits[ar, i2]
    g1 = 1.0 / (1.0 + np.exp(l2 - l1))
    g2 = 1.0 - g1
    idx_e, scl_e = [], []
    for e in range(E):
        m1 = i1 == e
        m2 = i2 == e
        ids = np.concatenate([np.nonzero(m1)[0], np.nonzero(m2)[0]])
        sc = np.concatenate([g1[m1], g2[m2]])
        idx_e.append(ids)
        scl_e.append(sc.astype(np.float32))
    return idx_e, scl_e


def prepare(x, gate_w, w1, w3, w2):
    """Host routing + sharding: returns (nc, in_maps, (idx_e, scl_e))."""
    import ml_dtypes

    xf = np.ascontiguousarray(x.reshape(T, D).astype(np.float32))
    xTb = np.ascontiguousarray(xf.T).astype(ml_dtypes.bfloat16)   # [D, T]

    idx_e, scl_e = _route(xf, gate_w)
    maxcnt = max(len(i) for i in idx_e)
    cap = ((maxcnt + 3) // 4) * 4     # 4-token grain keeps DMA rows 8B-aligned
    chunks = _chunks(cap)

    if cap not in _cache:
        _cache[cap] = _build_nc(cap)
    nc = _cache[cap]

    in_maps = []
    for c in range(NCORES):
        ids = idx_e[c]
        cnt = len(ids)
        xg = np.zeros((D, cap), dtype=ml_dtypes.bfloat16)
        xg[:, :cnt] = xTb[:, ids]
        # chunk-major, then k-major partition blocks: chunk rows contiguous
        xh = np.concatenate([
            xg[:, o:o + tcz].reshape(DK, P, tcz).transpose(1, 0, 2)
            .reshape(P, DK * tcz) for (o, tcz) in chunks], axis=1)

        w1T = np.ascontiguousarray(w1[c].T).astype(ml_dtypes.bfloat16)  # [D,F]
        w3T = np.ascontiguousarray(w3[c].T).astype(ml_dtypes.bfloat16)
        w2T = np.ascontiguousarray(w2[c].T).astype(ml_dtypes.bfloat16)  # [F,D]

        def fmaj(wT):
            # [D, F] -> [128, f-major [f][k][128]] per-f-tile blocks
            return np.concatenate([
                wT[:, f * P:(f + 1) * P].reshape(DK, P, P).transpose(1, 0, 2)
                .reshape(P, BLK) for f in range(FK)], axis=1)

        in_maps.append({
            "xh": np.ascontiguousarray(xh),
            "w1h": fmaj(w1T),
            "w3h": fmaj(w3T),
            "w2h": np.ascontiguousarray(
                w2T.reshape(FK, P, D).transpose(1, 0, 2).reshape(P, FK * D)),
        })
    return nc, in_maps, (idx_e, scl_e, chunks)


def _combine(res, meta):
    idx_e, scl_e, chunks = meta
    outf = np.zeros((T, D), dtype=np.float32)
    for c in range(NCORES):
        cnt = len(idx_e[c])
        raw = res.results[c]["out"].astype(np.float32)   # [128, 8*cap]
        cap = raw.shape[1] // DK
        y = np.empty((D, cap), dtype=np.float32)
        for (o, tcz) in chunks:
            blk = raw[:, DK * o:DK * (o + tcz)].reshape(P, 8, tcz)
            for dglob in range(8):
                y[dglob * P:(dglob + 1) * P, o:o + tcz] = blk[:, dglob, :]
        outf[idx_e[c]] += scl_e[c][:, None] * y[:, :cnt].T
    return outf.reshape(B, S, D)


def kernel(x, gate_w, w1, w3, w2):
    from concourse.bass_utils import run_bass_kernel_spmd

    nc, in_maps, meta = prepare(x, gate_w, w1, w3, w2)
    res = run_bass_kernel_spmd(nc, in_maps, list(range(NCORES)))
    return _combine(res, meta)
